# revision 50
# baseline (speedup 1.0000x reference)
"""GQA + sliding-window attention (B=2, S=2048, E=2048, HQ=16, HKV=4, D=128, W=512).

Sharding: 8 cores = 2 batches x 4 KV-head groups (tensor parallel).
Each core computes its batch's full sequence for one KV head + its 4 Q heads,
plus the (row-sharded) output projection partial; the host sums the 4 partials
per batch (the "all-reduce" done host-side) and adds bo.

On-device layout (per core):
  xT   [E, S]   bf16  (x[b] transposed on host)
  wqT  [E, 512] bf16  (Wq rows of this group, transposed)
  wkT  [E, 128] bf16  (pre-scaled by 1/sqrt(D))
  wvT  [E, 128] bf16
  woT  [512, E] bf16  (Wo cols of this group, transposed)
  cosT/sinT [128, S] f32 RoPE tables (sinT sign-folded for rotate-half)
  consts [128, 128] bf16: the rotate-half partition-swap permutation
  out  [S, E]   f32   partial output

Perf notes (sim-driven, 359us -> ~214us):
- input DMAs are batched (flat ~625ns HWDGE cost per dma_start) and x is
  loaded in S-column chunks so the projections stream one chunk behind the
  load instead of idling ~25us;
- the RoPE rotate-half partition swap is a permutation matmul instead of an
  SBUF->SBUF DMA (a DMA adds ~900ns sem latency to the dependency chain);
- causal/window masks are applied multiplicatively after exp (exp(s + m) ==
  exp(s)*mask) via affine_select on the otherwise-idle Pool engine;
- the softmax normalizer 1/rowsum is broadcast across partitions with a
  gpsimd partition_broadcast on the otherwise-idle Pool engine (attn gpsimd
  library), and applied one iteration later so no engine waits in-order on
  a cross-engine result;
- each po psum tile [128,512] packs PV accum | rowsum row | edge score
  tile, so scores(2) + po(6) fit the 8 psum banks exactly;
- the main score+exp run two iterations ahead of their PV consumers, and
  all V-projections run at the projection tail so the per-chunk K/Q stream
  never waits on the late-loading V weights;
- the out projection stages a full [128, E] row and writes it with one DMA
  (the last two rows in 512-col pieces to shorten the final drain).
"""

import os

import numpy as np
import ml_dtypes

import concourse.bass as bass
import concourse.library_config as library_config
import concourse.mybir as mybir
import concourse.tile as tile
from concourse.tile import add_dep_helper
from concourse.bass_utils import run_bass_kernel_spmd

B, S, E = 2, 2048, 2048
HQ, HKV, D = 16, 4, 128
WINDOW = 512
ROPE_BASE = 10000.0
N_CORES = 8
GROUP = HQ // HKV          # 4 Q heads per KV head
HD_Q = GROUP * D           # 512
ST = S // 128              # 16 sequence tiles
WT = WINDOW // 128         # 4 -> window spans WT+1 = 5 q-tiles
NEG = -30000.0

f32 = mybir.dt.float32
bf16 = mybir.dt.bfloat16


def _split_sync_waits(nc, max_waits=1):
    """walrus in this container rejects instructions with more than one
    sync-wait; split extras onto preceding same-engine NoOps."""
    for fn in nc.m.functions:
        for blk in fn.blocks:
            new_insts = []
            for inst in blk.instructions:
                si = getattr(inst, "sync_info", None)
                if si is not None and len(si.on_wait) > max_waits:
                    waits = list(si.on_wait)
                    head, tail = waits[:-max_waits], waits[-max_waits:]
                    for i in range(0, len(head), max_waits):
                        nop = mybir.InstNoOp(
                            name=f"splitwait-{nc.next_id()}",
                            ins=[], outs=[],
                            sync_info=mybir.SyncInfo(
                                on_wait=head[i:i + max_waits], on_update=[]),
                            bass_nofuse=True,
                        )
                        nop.engine = inst.engine
                        new_insts.append(nop)
                    inst.sync_info = mybir.SyncInfo(
                        on_wait=tail, on_update=list(si.on_update))
                new_insts.append(inst)
            blk.instructions[:] = new_insts


def build_kernel(has_bias):
    nc = bass.Bass("TRN2", target_bir_lowering=False, debug=False,
                   num_devices=N_CORES)
    Exp = mybir.ActivationFunctionType.Exp

    xT = nc.dram_tensor("xT", [E, S], bf16, kind="ExternalInput").ap()
    wqT = nc.dram_tensor("wqT", [E, HD_Q], bf16, kind="ExternalInput").ap()
    wkT = nc.dram_tensor("wkT", [E, D], bf16, kind="ExternalInput").ap()
    wvT = nc.dram_tensor("wvT", [E, D], bf16, kind="ExternalInput").ap()
    woT = nc.dram_tensor("woT", [HD_Q, E], bf16, kind="ExternalInput").ap()
    cosT = nc.dram_tensor("cosT", [D, S], f32, kind="ExternalInput").ap()
    sinT = nc.dram_tensor("sinT", [D, S], f32, kind="ExternalInput").ap()
    constsT = nc.dram_tensor("constsT", [D, D], bf16, kind="ExternalInput").ap()
    if has_bias:
        bqr = nc.dram_tensor("bqr", [1, HD_Q], bf16, kind="ExternalInput").ap()
        bkr = nc.dram_tensor("bkr", [1, D], bf16, kind="ExternalInput").ap()
        bvr = nc.dram_tensor("bvr", [1, D], bf16, kind="ExternalInput").ap()
    out = nc.dram_tensor("out", [S, E], f32, kind="ExternalOutput").ap()

    WARMUP = 48
    with tile.TileContext(nc) as tc:
        with tc.tile_pool(name="singles", bufs=1) as singles, \
             tc.tile_pool(name="upool", bufs=4) as upool, \
             tc.tile_pool(name="epool", bufs=6) as epool, \
             tc.tile_pool(name="rhatpool", bufs=4) as rhatpool, \
             tc.tile_pool(name="rbpool", bufs=4) as rbpool, \
             tc.tile_pool(name="ostage", bufs=3) as ostage:

            # ---- resident tensors ----
            xt = singles.tile([128, KTILES, S], bf16)
            wq = singles.tile([128, KTILES, HD_Q], bf16)
            wk = singles.tile([128, KTILES, D], bf16)
            wv = singles.tile([128, KTILES, D], bf16)
            wo = singles.tile([128, GROUP, E], bf16)
            cost = singles.tile([128, S], f32)
            sint = singles.tile([128, S], f32)
            consts = singles.tile([128, 128], bf16)
            qt = singles.tile([128, GROUP, S], bf16)
            kt = singles.tile([128, S], bf16)
            vv = singles.tile([128, ST, D], bf16)
            ot = singles.tile([128, GROUP * ST, D], bf16)
            onescol = singles.tile([128, 1], bf16)

            pswapb = consts[:, 0:128]

            # ---- batched input loads (order = DMA queue order) ----
            # x is loaded in S-column chunks: projections are local in S, so
            # compute streams one chunk behind the load instead of waiting
            # ~25us for the whole activation to land.
            xr = xT.rearrange("(k p) s -> p k s", p=128)
            NCH = 8
            CW = S // NCH                                  # 256 cols per chunk
            nc.sync.dma_start(out=wk[:], in_=wkT.rearrange("(k p) d -> p k d", p=128))
            nc.sync.dma_start(out=consts[:], in_=constsT)
            bq_t = bk_t = bv_t = onesrow = None
            if has_bias:
                bq_t = singles.tile([1, HD_Q], bf16)
                bk_t = singles.tile([1, D], bf16)
                bv_t = singles.tile([1, D], bf16)
                onesrow = singles.tile([1, 512], bf16)
                nc.sync.dma_start(out=bq_t[:], in_=bqr)
                nc.sync.dma_start(out=bk_t[:], in_=bkr)
                nc.sync.dma_start(out=bv_t[:], in_=bvr)
                nc.gpsimd.memset(onesrow[:], 1.0)
            nc.sync.dma_start(out=xt[:, :, 0:CW], in_=xr[:, :, 0:CW])
            nc.sync.dma_start(out=wv[:], in_=wvT.rearrange("(k p) d -> p k d", p=128))
            wqr = wqT.rearrange("(k p) m -> p k m", p=128)
            nc.sync.dma_start(out=wq[:, :, 0:256], in_=wqr[:, :, 0:256])
            nc.sync.dma_start(out=wq[:, :, 256:HD_Q], in_=wqr[:, :, 256:HD_Q])
            nc.sync.dma_start(out=cost[:], in_=cosT)
            nc.sync.dma_start(out=sint[:], in_=sinT)
            for c in range(1, NCH):
                nc.sync.dma_start(out=xt[:, :, c * CW:(c + 1) * CW],
                                  in_=xr[:, :, c * CW:(c + 1) * CW])
            nc.sync.dma_start(out=wo[:], in_=woT.rearrange("(h p) e -> p h e", p=128))

            nc.gpsimd.memset(onescol[:], 1.0)
            # partition_broadcast lives in the attn gpsimd library; load it
            # once up front while Pool is otherwise idle
            nc.gpsimd.load_library(library_config.attn)

            with tc.tile_pool(name="qk_psum", bufs=2, space="PSUM") as qk_psum, \
                 tc.tile_pool(name="swap_psum", bufs=2, space="PSUM") as swap_psum, \
                 tc.tile_pool(name="v_psum", bufs=2, space="PSUM") as v_psum:

                # rope finish pipeline state, shared across all rope jobs:
                # dst = ps*cos + Pswap @ (ps*sin'), partition swap on the PE
                # (no DMA in the chain); each job's finish is emitted after
                # the next job's matmuls so DVE never stalls the PE.
                prev = [None]

                def rope_finish():
                    ps_p, u_p, dst, sl_p = prev[0]
                    ps2 = swap_psum.tile([128, CW], f32, tag="ps2")
                    nc.tensor.matmul(ps2[:], pswapb, u_p[:],
                                     start=True, stop=True)
                    nc.vector.tensor_mul(dst[:, sl_p], ps_p[:], cost[:, sl_p])
                    nc.vector.tensor_add(dst[:, sl_p], dst[:, sl_p], ps2[:])
                    prev[0] = None

                def proj_rope_chunk(dst, wtile, m_off, btile, n):
                    """dst[:, n*CW:(n+1)*CW] = rope chunk of W^T.T @ x^T."""
                    ps = qk_psum.tile([128, CW], f32, tag="ps")
                    for k in range(KTILES):
                        nc.tensor.matmul(
                            ps[:], wtile[:, k, m_off:m_off + 128],
                            xt[:, k, n * CW:(n + 1) * CW],
                            start=(k == 0),
                            stop=(k == KTILES - 1 and btile is None))
                    if btile is not None:
                        nc.tensor.matmul(
                            ps[:], btile[0:1, m_off:m_off + 128],
                            onesrow[0:1, 0:CW], start=False, stop=True)
                    if prev[0] is not None:
                        rope_finish()
                    u_t = upool.tile([128, CW], bf16, tag="u")
                    sl = slice(n * CW, (n + 1) * CW)
                    nc.vector.tensor_mul(u_t[:], ps[:], sint[:, sl])
                    prev[0] = (ps, u_t, dst, sl)

                SMC = CW // 128                        # V s-tiles per chunk
                for c in range(NCH):
                    # K^T projection + rope (pre-scaled by 1/sqrt(D) on host)
                    proj_rope_chunk(kt, wk, 0, bk_t, c)
                    # V in natural [s, d] layout (no rope)
                    for sm in range(SMC * c, SMC * c + SMC):
                        psv = v_psum.tile([128, 128], f32, tag="vps")
                        for k in range(KTILES):
                            nc.tensor.matmul(
                                psv[:], xt[:, k, sm * 128:(sm + 1) * 128],
                                wv[:, k, :], start=(k == 0),
                                stop=(k == KTILES - 1 and not has_bias))
                        if has_bias:
                            nc.tensor.matmul(
                                psv[:], onesrow[0:1, 0:128],
                                bv_t[0:1, :], start=False, stop=True)
                        nc.scalar.copy(vv[:, sm, :], psv[:])
                    # flat pipeline over (head, k-tile): head m+1's score work
                # fills head m's drain bubbles; finish ladder spans heads
                e_tiles = {}
                po_tiles = {}
                pv0 = {}
                rhat_t = {}
                rb_t = {}
                rb_pending = []
                mul_pending = []

                def emit_rb(key):
                    # broadcast 1/rowsum across partitions on the (idle)
                    # Pool engine -- SBUF-only, no PE/DVE cost
                    rb = rbpool.tile([128, 128], bf16, tag="rb")
                    nc.gpsimd.partition_broadcast(rb[:], rhat_t.pop(key)[:])
                    rb_t[key] = rb
                    mul_pending.append(key)

                def emit_mul(key):
                    po = po_tiles.pop(key)
                    pv0.pop(key, None)
                    m_, qi_ = key
                    nc.vector.tensor_mul(
                        ot[:, m_ * ST + qi_, :], po[:, 0:128], rb_t.pop(key)[:])

                def contrib(m, kj):
                    """PV + row-sum contributions of E_(m,kj); the deferred
                    normalization ladder (recip@contrib(qi), broadcast+mul one
                    iteration later) keeps every engine off the critical path
                    of any other."""
                    e_t = e_tiles.pop((m, kj))
                    qis = list(range(kj, min(kj + WT, ST - 1) + 1))
                    for qi in qis:
                        first = (kj == max(0, qi - WT))
                        if first and (m, qi) not in po_tiles:
                            po_tiles[(m, qi)] = pv_psum.tile(
                                [128, 512], f32, tag="po",
                                name=f"po_{m}_{qi}")
                        off = (qi - kj) * 128
                        mm = nc.tensor.matmul(
                            po_tiles[(m, qi)][:, 0:128], vv[:, kj, :],
                            e_t[:, off:off + 128],
                            start=first, stop=(qi == kj))
                        if first:
                            pv0[(m, qi)] = mm
                    for qi in qis:
                        first = (kj == max(0, qi - WT))
                        off = (qi - kj) * 128
                        mm = nc.tensor.matmul(
                            po_tiles[(m, qi)][0:1, 128:256], onescol[:],
                            e_t[:, off:off + 128],
                            start=False, stop=(qi == kj),
                            skip_group_check=True)
                        if first:
                            # rT group relies on pv0's start=True having
                            # cleared the bank's has_written bits first
                            add_dep_helper(mm.ins, pv0[(m, qi)].ins, sync=False,
                                           reason="rT after bank clear")
                    rhat = rhatpool.tile([1, 128], bf16, tag="rhat")
                    with nc.allow_low_precision(
                            reason="bf16 softmax denominator (~0.4% rel)"):
                        nc.vector.reciprocal(
                            rhat[:], po_tiles[(m, kj)][0:1, 128:256])
                    rhat_t[(m, kj)] = rhat
                    rb_pending.append((m, kj))

                prev_g = None
                for g in range(GROUP * ST):
                    m, kj = divmod(g, ST)
                    nw = min(WT + 1, ST - kj)
                    nmain = min(nw, WT) * 128
                    q0 = kj * 128
                    ksl = slice(kj * 128, (kj + 1) * 128)
                    # deferred finish from two iterations back; frees the
                    # po slot the edge-score below will reuse
                    if rb_pending:
                        emit_rb(rb_pending.pop(0))
                    if mul_pending:
                        emit_mul(mul_pending.pop(0))
                    # main score + causal mask (PE accumulation) + exp,
                    # issued a full iteration before their PV consumers
                    pss = score_psum.tile([128, 512], f32, tag="ss")
                    nc.tensor.matmul(
                        pss[:, 0:nmain], kt[:, ksl],
                        qt[:, m, q0:q0 + nmain], start=True, stop=True)
                    nc.tensor.matmul(
                        pss[:, 0:128], identb, m0b,
                        start=False, stop=True, skip_group_check=True)
                    e_t = epool.tile([128, 640], bf16, tag="e")
                    nc.scalar.activation(e_t[:, 0:nmain], pss[:, 0:nmain], Exp)
                    if nw == WT + 1:
                        # 5th (window-edge) score tile lives in the po
                        # psum bank of qi = kj+4
                        qi5 = kj + WT
                        po5 = pv_psum.tile([128, 512], f32, tag="po",
                                           name=f"po_{m}_{qi5}")
                        po_tiles[(m, qi5)] = po5
                        nc.tensor.matmul(
                            po5[:, 256:384], kt[:, ksl],
                            qt[:, m, (qi5) * 128:(qi5 + 1) * 128],
                            start=True, stop=True)
                        nc.tensor.matmul(
                            po5[:, 256:384], identb, m4b,
                            start=False, stop=True, skip_group_check=True)
                        nc.scalar.activation(e_t[:, 512:640],
                                             po5[:, 256:384], Exp)
                    if prev_g is not None:
                        contrib(*prev_g)
                    e_tiles[(m, kj)] = e_t
                    prev_g = (m, kj)
                # drain: last contributions + deferred finishes
                while rb_pending:
                    emit_rb(rb_pending.pop(0))
                while mul_pending:
                    emit_mul(mul_pending.pop(0))
                contrib(*prev_g)
                while rb_pending:
                    emit_rb(rb_pending.pop(0))
                while mul_pending:
                    emit_mul(mul_pending.pop(0))

            # output projection: out[q, :] = sum_h O_h[q, :] @ WoT_h
            with tc.tile_pool(name="out_psum", bufs=4, space="PSUM") as out_psum:
                for qi in range(ST):
                    st = ostage.tile([128, E], f32, tag="st")
                    for ch in range(E // 512):
                        pso = out_psum.tile([128, 512], f32, tag="po2")
                        for h in range(GROUP):
                            nc.tensor.matmul(
                                pso[:], ot[:, h * ST + qi, :],
                                wo[:, h, ch * 512:(ch + 1) * 512],
                                start=(h == 0), stop=(h == GROUP - 1))
                        if ch % 2 == 0:
                            nc.vector.tensor_copy(
                                st[:, ch * 512:(ch + 1) * 512], pso[:])
                        else:
                            nc.scalar.copy(st[:, ch * 512:(ch + 1) * 512], pso[:])
                        if qi >= ST - 2:
                            # drain the final rows in 512-col pieces so the
                            # last DMA isn't a 2.9us monolith at the very end
                            nc.sync.dma_start(
                                out=out[qi * 128:(qi + 1) * 128,
                                        ch * 512:(ch + 1) * 512],
                                in_=st[:, ch * 512:(ch + 1) * 512])
                    if qi < ST - 2:
                        nc.sync.dma_start(
                            out=out[qi * 128:(qi + 1) * 128, :], in_=st[:])

    _split_sync_waits(nc)
    return nc


def _rope_tables():
    half = D // 2
    inv_freq = 1.0 / (ROPE_BASE ** (np.arange(half, dtype=np.float64) / half))
    ang = np.arange(S, dtype=np.float64)[:, None] * inv_freq[None, :]  # [S, 64]
    cos = np.cos(ang).T.astype(np.float32)          # [64, S]
    sin = np.sin(ang).T.astype(np.float32)
    cosT = np.concatenate([cos, cos], 0)            # [128, S]
    sinT = np.concatenate([sin, -sin], 0)           # sign-folded rotate-half
    return np.ascontiguousarray(cosT), np.ascontiguousarray(sinT)


def _consts():
    """Rotate-half partition-swap permutation matrix, bf16-exact."""
    pswap = np.zeros((128, 128), np.float32)
    for p in range(128):
        pswap[(p + 64) % 128, p] = 1.0
    return pswap


def kernel(x, Wq, bq, Wk, bk, Wv, bv, Wo, bo, **kw):
    x = np.asarray(x, np.float32)
    Wq = np.asarray(Wq, np.float32); bq = np.asarray(bq, np.float32)
    Wk = np.asarray(Wk, np.float32); bk = np.asarray(bk, np.float32)
    Wv = np.asarray(Wv, np.float32); bv = np.asarray(bv, np.float32)
    Wo = np.asarray(Wo, np.float32); bo = np.asarray(bo, np.float32)

    has_bias = bool(np.any(bq) or np.any(bk) or np.any(bv))
    nc = build_kernel(has_bias)

    bff = ml_dtypes.bfloat16
    cosT, sinT = _rope_tables()
    consts = _consts().astype(bff)
    scale = 1.0 / np.sqrt(np.float32(D))

    in_maps = []
    for c in range(N_CORES):
        b, h = c // HKV, c % HKV
        qs = slice(h * HD_Q, (h + 1) * HD_Q)
        ks = slice(h * D, (h + 1) * D)
        m = {
            "xT": np.ascontiguousarray(x[b].T).astype(bff),
            "wqT": np.ascontiguousarray(Wq[qs].T).astype(bff),
            "wkT": np.ascontiguousarray((Wk[ks] * scale).T).astype(bff),
            "wvT": np.ascontiguousarray(Wv[ks].T).astype(bff),
            "woT": np.ascontiguousarray(Wo[:, qs].T).astype(bff),
            "cosT": cosT,
            "sinT": sinT,
            "constsT": consts,
        }
        if has_bias:
            m["bqr"] = np.ascontiguousarray(bq[qs][None, :]).astype(bff)
            m["bkr"] = np.ascontiguousarray((bk[ks] * scale)[None, :]).astype(bff)
            m["bvr"] = np.ascontiguousarray(bv[ks][None, :]).astype(bff)
        in_maps.append(m)

    res = run_bass_kernel_spmd(nc, in_maps, core_ids=list(range(N_CORES)))
    global LAST_RESULT
    LAST_RESULT = res
    if os.environ.get("BASS_KERNEL_RETIME"):
        # executable is now cached in-process: a second run times
        # transfer + device execution without compile.
        import time
        t0 = time.time()
        run_bass_kernel_spmd(nc, in_maps, core_ids=list(range(N_CORES)))
        print(f"retime run (transfer+exec): {time.time()-t0:.3f}s")

    out_full = np.zeros((B, S, E), np.float32)
    for c in range(N_CORES):
        out_full[c // HKV] += res.results[c]["out"]
    out_full += bo[None, None, :]
    return out_full
